# revision 11
# baseline (speedup 1.0000x reference)
"""Trainium2 Bass kernel: batched single-channel 7x7 conv2d (stride 1, pad 3).

Strategy
--------
Pure data parallel over batch: 64 images -> 8 cores x 8 images.

Per core, the 2D conv is computed on the TensorEngine as 7 accumulating
matmuls per output tile: for each horizontal tap v, a banded-Toeplitz
matrix T_v ([K=128 input rows, M<=122 output rows], T_v[k,m] = W[u,v]
with u = d + k - m) performs the full 7-tap *vertical* convolution of a
128-row image strip in one matmul; the 7 horizontal taps come from
column-shifted access patterns on the same SBUF strip, accumulated in
PSUM. Zero padding is realized by clipping the Toeplitz band (rows) and
by narrowing the out/rhs column ranges (columns) - no zero-fill needed.

Row tiling: each 128-row strip yields 122 complete output rows; 9 strips
cover a 1024-row image. Three Toeplitz variants (band offsets d = 3, 0,
-77) handle the first / interior / last strips.

Inputs are cast to bf16 on host (PSUM accumulates fp32); output is fp32.
"""

import os
import numpy as np
import ml_dtypes
from contextlib import ExitStack

import concourse.bass as bass
import concourse.tile as tile
from concourse import bacc, mybir
from concourse.bass_utils import run_bass_kernel_spmd

N_CORES = 8
B, H, W_IMG = 64, 1024, 1024
B_LOC = B // N_CORES
KS, PAD = 7, 3
TILE_ROWS = 128
# A 128-row input strip yields up to 122 complete output rows, but SBUF->DRAM
# stores only spread across all 16 SDMA engines when the partition count is a
# multiple of 8 (measured: 122/124-partition stores pin to 2 engines at
# ~50 GB/s; 120/112/96/64 run at ~400 GB/s). 120 keeps the same strip count.
OUT_ROWS = 120
COL_BLOCK = 512


def row_tiles(h):
    """Per-image row tiling: list of (A, O, M, d).

    A: first input row loaded (128 rows [A, A+128) always in-bounds),
    O: first output row, M: number of output rows, d: Toeplitz band
    offset (= A - O + PAD).
    """
    tiles = []
    o = 0
    while o < h:
        m = min(OUT_ROWS, h - o)
        a = min(max(o - PAD, 0), h - TILE_ROWS)
        tiles.append((a, o, m, a - o + PAD))
        o += m
    return tiles


def col_blocks(w):
    blocks = []
    c = 0
    while c < w:
        blocks.append((c, min(c + COL_BLOCK, w)))
        c += COL_BLOCK
    return blocks


def tap_ranges(c0, c1, w):
    """For each tap v: (out_lo, out_hi, shift) with rhs cols = out cols + shift."""
    out = []
    for v in range(KS):
        sh = v - PAD
        lo = max(c0, -sh)
        hi = min(c1, w - sh)
        out.append((lo, hi, sh))
    return out


def build_toeplitz(w7, d_list, np_dtype):
    """Packed Toeplitz weights [128, len(d_list)*7*128].

    Slice [:, (di*7+v)*128:(di*7+v+1)*128][k, m] = W[d+k-m, v] (0 if out
    of band). Column m of slice (di, v) is output row m of a strip with
    band offset d = d_list[di].
    """
    n = len(d_list)
    t = np.zeros((TILE_ROWS, n * KS, TILE_ROWS), dtype=np.float32)
    k = np.arange(TILE_ROWS)[:, None]
    m = np.arange(TILE_ROWS)[None, :]
    for di, d in enumerate(d_list):
        u = d + k - m
        mask = (u >= 0) & (u < KS)
        uu = np.clip(u, 0, KS - 1)
        for v in range(KS):
            t[:, di * KS + v, :] = np.where(mask, w7[uu, v], 0.0)
    return np.ascontiguousarray(
        t.reshape(TILE_ROWS, n * KS * TILE_ROWS).astype(np_dtype)
    )


def build_program(b_loc, h, w, in_dt=mybir.dt.bfloat16):
    """Build + compile the per-core Bass program. Returns (nc, d_list)."""
    tiles = row_tiles(h)
    blocks = col_blocks(w)
    d_list = sorted({d for (_, _, _, d) in tiles})
    d_idx = {d: i for i, d in enumerate(d_list)}
    n_toep = len(d_list) * KS

    nc = bacc.Bacc("TRN2", target_bir_lowering=False, debug=False)
    x_d = nc.dram_tensor("x", [b_loc, h, w], in_dt, kind="ExternalInput").ap()
    t_d = nc.dram_tensor(
        "toep", [TILE_ROWS, n_toep * TILE_ROWS], in_dt, kind="ExternalInput"
    ).ap()
    y_d = nc.dram_tensor("y", [b_loc, h, w], mybir.dt.float32, kind="ExternalOutput").ap()

    with tile.TileContext(nc) as tc, ExitStack() as ctx:
        wpool = ctx.enter_context(tc.tile_pool(name="wpool", bufs=1))
        inpool = ctx.enter_context(tc.tile_pool(name="inpool", bufs=6))
        outpool = ctx.enter_context(tc.tile_pool(name="outpool", bufs=6))
        pspool = ctx.enter_context(tc.tile_pool(name="pspool", bufs=8, space="PSUM"))

        wt = wpool.tile([TILE_ROWS, n_toep * TILE_ROWS], in_dt, name="wt")
        nc.sync.dma_start(wt[:], t_d[:])

        # fp32r matmuls require even PSUM free-dim offset/size, which the
        # ragged edge-tap ranges violate; pad 4 zero columns each side so
        # every tap is full-width instead.
        padc = 4 if in_dt == mybir.dt.float32r else 0

        # Pre-warm the PE's HAM clock gate during the initial DMA head so the
        # first real matmuls run at 2.4 GHz instead of 1.2 GHz.
        warm = wpool.tile([TILE_ROWS, COL_BLOCK], in_dt, name="warm")
        nc.gpsimd.memset(warm[:].bitcast(mybir.dt.float32), 0.0)
        wps = pspool.tile([TILE_ROWS, COL_BLOCK], mybir.dt.float32, name="wps", tag="ps")
        for i in range(24):
            nc.tensor.matmul(
                wps[:],
                warm[:, :TILE_ROWS],
                warm[:],
                start=(i == 0),
                stop=(i == 23),
            )

        for bi in range(b_loc):
            for (a, o, m, d) in tiles:
                xt = inpool.tile([TILE_ROWS, w + 2 * padc], in_dt, name="xt", tag="xt")
                if padc:
                    # memset rejects the float32r value type; zero the pad
                    # columns through a plain-f32 view of the same bytes
                    nc.gpsimd.memset(xt[:, :padc].bitcast(mybir.dt.float32), 0.0)
                    nc.gpsimd.memset(xt[:, w + padc :].bitcast(mybir.dt.float32), 0.0)
                # loads on the scalar HWDGE ring, stores on sync's: two FIFOs
                nc.scalar.dma_start(xt[:, padc : w + padc], x_d[bi, a : a + TILE_ROWS, :])
                ot = outpool.tile([TILE_ROWS, w], mybir.dt.float32, name="ot", tag="ot")
                pss = [
                    pspool.tile(
                        [TILE_ROWS, COL_BLOCK], mybir.dt.float32, name="ps", tag="ps"
                    )
                    for _ in blocks
                ]
                # tap-major: both column blocks reuse one lhsT back-to-back
                order = [PAD] + [v for v in range(KS) if v != PAD]
                for i, v in enumerate(order):
                    lhsT = wt[:, (d_idx[d] * KS + v) * TILE_ROWS :][:, :m]
                    for cb, (c0, c1) in enumerate(blocks):
                        cw = c1 - c0
                        if padc:
                            lo, hi, sh = c0, c1, v - PAD
                            out_ap = pss[cb][:m, :cw]
                        else:
                            lo, hi, sh = tap_ranges(c0, c1, w)[v]
                            out_ap = pss[cb][:m, lo - c0 : hi - c0]
                        nc.tensor.matmul(
                            out_ap,
                            lhsT,
                            xt[:, padc + lo + sh : padc + hi + sh],
                            start=(i == 0),
                            stop=(i == KS - 1),
                        )
                for cb, (c0, c1) in enumerate(blocks):
                    cw = c1 - c0
                    if cb % 2 == 0:
                        nc.vector.tensor_copy(ot[:m, c0:c1], pss[cb][:m, :cw])
                    else:
                        nc.scalar.copy(ot[:m, c0:c1], pss[cb][:m, :cw])
                nc.sync.dma_start(y_d[bi, o : o + m, :], ot[:m, :])

    nc.compile()
    return nc, d_list


_CACHE = {}


def _get_program(b_loc, h, w, in_dt):
    key = (b_loc, h, w, in_dt)
    if key not in _CACHE:
        _CACHE[key] = build_program(b_loc, h, w, in_dt=in_dt)
    return _CACHE[key]


# Measured on HW (full 64x1024x1024 problem, 8 cores):
#   bfloat16: 240 us/core, rel err 2.0e-3
#   float32r: 256 us/core, rel err 1.2e-4  (fp32 storage, TF32-like matmul,
#             1 cycle/row at N>=256 - near the bf16 PE rate)
# float32r default: 16x better accuracy for ~7% more time.
IN_DT = mybir.dt.float32r


def kernel(X, W, _trace=False, _trace_dir=None):
    X = np.asarray(X, dtype=np.float32)
    W = np.asarray(W, dtype=np.float32)
    assert X.shape == (B, H, W_IMG) and W.shape == (KS, KS)

    nc, d_list = _get_program(B_LOC, H, W_IMG, IN_DT)
    np_dt = mybir.dt.np(IN_DT)
    x_cast = X.astype(np_dt) if np_dt != np.float32 else X
    toep = build_toeplitz(W, d_list, np_dt)
    in_maps = [
        {"x": x_cast[c * B_LOC : (c + 1) * B_LOC], "toep": toep}
        for c in range(N_CORES)
    ]
    res = run_bass_kernel_spmd(
        nc, in_maps, list(range(N_CORES)), trace=_trace, tmpdir=_trace_dir
    )
    out = np.concatenate([res.results[c]["y"] for c in range(N_CORES)], axis=0)
    if _trace:
        return out, res
    return out
